# revision 23
# baseline (speedup 1.0000x reference)
"""RelGraphConv (3-layer, 2-relation) GNN message passing on 8 trn2 NeuronCores.

Strategy: partition nodes across cores (graph parallel). Per layer, each core
gathers raw source-node features for its incoming edges (dma_gather from a
replicated HBM feature table), aggregates per (dst, relation) slot with
one-hot matmuls accumulated in PSUM, applies the per-relation weights after
aggregation (the conv is linear, so W can be applied post-aggregation), and
AllGathers the new node features into the next layer's table.

Implementation notes (hardware-driven):
- The DMA engines cost ~constant time per gather descriptor up to ~512B.
  Layer 0 gathers direct 256B rows (x is [N,128] fp16) with a two-range src
  split (dma_gather idx is int16: rows >= 32768 gather from a base-offset
  view). Layers 1/2 (64 feats = 128B < the 256B DMA minimum) keep
  pair-packed 256B rows with parity-masked one-hot S matmuls.
- The inter-layer AllGather is split at HA nodes per core, and layer l+1's
  edge stream is split per block into region-A tiles (src in the first
  AllGather half) and region-B tiles. All A gathers of the whole layer run
  as phase A (they only need the early collective), then phase B gathers
  the rest and finishes each block: this removes the Pool idle bubble at
  layer boundaries (the collectives' DMA traffic overlaps phase A/B).
- Tiles are padded per (block, region); units are (tile, chunk, parity)
  runs taken as a union over cores, masked per-core via oo=255 columns.
- dma_gather wedges above 1024 indices per call -> sub-gathers of <= 8
  tiles round-robined over 4 SWDGE queues. S matrices are built 8 tiles
  per DVE op via step-0 broadcast APs. fp16 feature path; PSUM fp32.
"""
import sys

sys.path.insert(0, "/opt/trn_rl_repo")

import numpy as np

import concourse.bacc as bacc
import concourse.bass as bass
import concourse.bass_isa as bass_isa
import concourse.tile as tile
from concourse import mybir
from concourse.bass_utils import run_bass_kernel_spmd

F32 = mybir.dt.float32
F16 = mybir.dt.float16
I16 = mybir.dt.int16
U8 = mybir.dt.uint8
AOT = mybir.AluOpType

GMAX = 8   # tiles per dma_gather (1024 idx hardware limit)
SBK = 8    # tiles per batched S-build
NQ = 4     # SWDGE queues
HA_BLOCKS = 19  # blocks per core in the first AllGather half
R0 = 32768      # L0 direct-gather low range size (int16 idx limit)


class Cfg:
    def __init__(self, N, E, feats, n_cores=8):
        self.N = N
        self.E = E
        self.feats = feats          # [F0, F1, F2, F3]
        self.n_cores = n_cores
        self.NL = N // n_cores      # nodes per core (must divide)
        assert self.NL * n_cores == N
        assert N % 2 == 0 and N // 2 < 32768
        # pad local nodes to blocks of 256 (= 4 chunks of 128 slots)
        self.NLP = ((self.NL + 255) // 256) * 256
        self.blocks = self.NLP // 256
        self.chunks = self.blocks * 4
        self.HA = min(HA_BLOCKS * 256, self.NL)   # first-half node count
        assert self.HA % 2 == 0 and (self.NL - self.HA) % 2 == 0


class SplitPlan:
    """Edge-stream plan with a per-(block, region) tile split.

    Regions: 0 = first table region (or low src range for L0), 1 = rest.
    Per block: region-0 tiles (edges sorted by (chunk, parity, ...)) then
    region-1 tiles. One matmul unit per (tile, chunk, parity) run present on
    any core. Calls are per (block, region) in <= GMAX-tile pieces.
    """

    def __init__(self, cfg, edges_key, n_cores):
        # edges_key: (core, blk, region, chunk4, parity, eidx) rows [E, 6]
        self.cfg = cfg
        E = len(edges_key)
        core, blk, regn, ch4, par = (edges_key[:, k] for k in range(5))
        blocks, chunks = cfg.blocks, cfg.chunks
        # counts per (core, blk, region)
        cbr = (core * blocks + blk) * 2 + regn
        cnt_cbr = np.bincount(cbr, minlength=n_cores * blocks * 2).reshape(
            n_cores, blocks, 2)
        cap = np.ceil(cnt_cbr.max(axis=0) / 128).astype(np.int64)  # [blk, 2]
        cap[(cap.sum(axis=1) == 0), 0] = 1
        self.cap = cap
        # global tile offsets: block-major, region 0 tiles then region 1
        self.t_off = np.zeros((blocks, 2), dtype=np.int64)
        pos = 0
        for b in range(blocks):
            self.t_off[b, 0] = pos
            pos += cap[b, 0]
            self.t_off[b, 1] = pos
            pos += cap[b, 1]
        self.n_tiles = pos

        # per-core edge position within its (core, blk, region) stream,
        # ordered by (chunk4, parity, eidx)
        order = np.lexsort((edges_key[:, 5], par, ch4, regn, blk, core))
        base_cbr = np.zeros(n_cores * blocks * 2, dtype=np.int64)
        np.cumsum(cnt_cbr.reshape(-1)[:-1], out=base_cbr[1:])
        pos_in = np.arange(E) - base_cbr[cbr[order]]
        gt = self.t_off[blk[order], regn[order]] + pos_in // 128
        self.order = order
        self.gtile = gt
        self.slot_pp = pos_in % 128

        # units: (tile, parity) pairs present on any core, tile-major.
        # A tile's rows may span chunks; the chunk is encoded per unit for
        # the matmul's pa target. unit key = (gt, chunk, parity).
        ukeys = np.unique(
            np.stack([gt, ch4[order] + 4 * blk[order], par[order]], axis=1),
            axis=0)
        # order units by (tile, chunk, parity) -> contiguous per block
        uord = np.lexsort((ukeys[:, 2], ukeys[:, 1], ukeys[:, 0]))
        ukeys = ukeys[uord]
        self.n_units = len(ukeys)
        self.umap = {tuple(k): i for i, k in enumerate(ukeys)}
        # per (chunk, region): ordered list of (gt, parity, unit)
        self.units_cr = {}
        for i, (g_, c_, p_) in enumerate(ukeys):
            r_ = 1 if g_ >= self.t_off[c_ // 4, 1] else 0
            self.units_cr.setdefault((int(c_), r_), []).append(
                (int(g_), int(p_), i))
        # unit range per block (for S builds): units are tile-major and tiles
        # are block-major, so each (block, region) owns a contiguous range.
        self.ublk = []
        for b in range(blocks):
            lo = self.t_off[b, 0]
            hi = self.t_off[b, 1] + cap[b, 1]
            us = np.searchsorted(ukeys[:, 0], lo, side="left")
            mid = np.searchsorted(ukeys[:, 0], self.t_off[b, 1], side="left")
            ue = np.searchsorted(ukeys[:, 0], hi, side="left")
            self.ublk.append((int(us), int(mid), int(ue)))
        # calls per (block, region)
        self.calls = []
        for b in range(blocks):
            cl = [[], []]
            for r in range(2):
                n = cap[b, r]
                st = self.t_off[b, r]
                for s in range(0, n, GMAX):
                    cl[r].append((int(st + s), int(min(GMAX, n - s))))
            self.calls.append(cl)

    def u_of_edges(self, blk_o, ch4_o, par_o):
        gt = self.gtile
        keys = np.stack([gt, ch4_o + 4 * blk_o, par_o], axis=1)
        return np.array([self.umap[tuple(k)] for k in keys], dtype=np.int64)


def preprocess(cfg, x, src, dst, etypes, cell_size, max_size):
    n_cores, NL, NLP, HA = cfg.n_cores, cfg.NL, cfg.NLP, cfg.HA
    HB = NL - HA
    E = len(src)
    core_of = dst // NL
    o = 2 * (dst - core_of * NL) + etypes
    blk = o // 512
    ch4 = (o // 128) % 4
    oo = (o % 128).astype(np.int64)

    # ---------- L0: direct rows, regions = src ranges, no parity ----------
    rng0 = (src >= R0).astype(np.int64)
    ek0 = np.stack([core_of, blk, rng0, ch4,
                    np.zeros(E, dtype=np.int64), np.arange(E)], axis=1)
    plan0 = SplitPlan(cfg, ek0, n_cores)
    idxv0 = np.where(rng0 == 0, src, src - R0).astype(np.int16)
    o0 = plan0.order
    u0 = plan0.u_of_edges(blk[o0], ch4[o0], np.zeros(E, dtype=np.int64))

    NI0 = plan0.n_tiles * 128
    NU0 = plan0.n_units
    idx0_arrs, oo0_arrs = [], []
    for c in range(n_cores):
        sel = core_of[o0] == c
        ia = np.zeros(NI0, dtype=np.int16)
        ia[plan0.gtile[sel] * 128 + plan0.slot_pp[sel]] = idxv0[o0][sel]
        idx0_arrs.append(np.tile(ia.reshape(NI0 // 16, 16).T, (8, 1)))
        ou = np.full((128, NU0), 255.0, dtype=np.float16)
        ou[plan0.slot_pp[sel], u0[sel]] = oo[o0][sel].astype(np.float16)
        oo0_arrs.append(ou)

    # ---------- L1/L2: pair rows, regions = AllGather halves ----------
    score = src // NL
    soff = src - score * NL
    g = np.where(soff < HA,
                 score * HA + soff,
                 n_cores * HA + score * HB + (soff - HA))
    idxv12 = (g >> 1).astype(np.int16)
    par = (src & 1).astype(np.int64)
    regn = (soff >= HA).astype(np.int64)
    rowsA = n_cores * HA // 2
    # region-1 gathers use a base offset of rowsA pair rows
    idxv12 = np.where(regn == 0, idxv12, idxv12 - rowsA).astype(np.int16)

    ek = np.stack([core_of, blk, regn, ch4, par, np.arange(E)], axis=1)
    plan12 = SplitPlan(cfg, ek, n_cores)
    o12 = plan12.order
    u12 = plan12.u_of_edges(blk[o12], ch4[o12], par[o12])

    NI = plan12.n_tiles * 128
    NU = plan12.n_units
    idx12_arrs, oo12_arrs = [], []
    for c in range(n_cores):
        sel = core_of[o12] == c
        ia = np.zeros(NI, dtype=np.int16)
        ia[plan12.gtile[sel] * 128 + plan12.slot_pp[sel]] = idxv12[o12][sel]
        idx12_arrs.append(np.tile(ia.reshape(NI // 16, 16).T, (8, 1)))
        ou = np.full((128, NU), 255.0, dtype=np.float16)
        ou[plan12.slot_pp[sel], u12[sel]] = oo[o12][sel].astype(np.float16)
        oo12_arrs.append(ou)

    xT, maskC, minmask = [], [], []
    for c in range(n_cores):
        xl = x[c * NL:(c + 1) * NL]
        xt = np.zeros((cfg.feats[0], NLP), dtype=np.float16)
        xt[:, :NL] = xl.T.astype(np.float16)
        xT.append(xt)
        cs = cell_size[c * NL:(c + 1) * NL]
        ms = max_size[c * NL:(c + 1) * NL]
        m = np.zeros((NLP, 2), dtype=np.float32)
        m[:NL, 0] = cs >= (ms - 1)
        m[:NL, 1] = cs == 0
        mm = np.zeros((NLP, 2), dtype=np.float32)
        mm[NL:, :] = 1e30
        nch = NLP // 128
        maskC.append(m.reshape(nch, 128, 2).transpose(1, 0, 2)
                     .reshape(128, nch * 2).astype(np.uint8))
        minmask.append(mm.reshape(nch, 128, 2).transpose(1, 0, 2)
                       .reshape(128, nch * 2).copy())

    return plan0, plan12, idx0_arrs, oo0_arrs, idx12_arrs, oo12_arrs, \
        xT, maskC, minmask


def build_program(cfg, plan0, plan12):
    F0, F1, F2, F3 = cfg.feats
    NLP, NL, HA = cfg.NLP, cfg.NL, cfg.HA
    NT0, NT = plan0.n_tiles, plan12.n_tiles
    NP = cfg.N // 2
    nch = NLP // 128
    n_cores = cfg.n_cores
    rowsA = n_cores * HA // 2

    nc = bacc.Bacc(None, target_bir_lowering=False, debug=False,
                   num_devices=cfg.n_cores, num_swdge_queues=NQ,
                   dynamic_dma_scratch_size=32768)

    xd_ext = nc.dram_tensor("xdir", [cfg.N, F0], F16, kind="ExternalInput")
    xT_ext = nc.dram_tensor("xT", [F0, NLP], F16, kind="ExternalInput")
    idx0_ext = nc.dram_tensor("idx0", [128, NT0 * 8], I16, kind="ExternalInput")
    idx12_ext = nc.dram_tensor("idx12", [128, NT * 8], I16, kind="ExternalInput")
    NU0, NU = plan0.n_units, plan12.n_units
    oo0_ext = nc.dram_tensor("oo0", [128, NU0], F16, kind="ExternalInput")
    oo12_ext = nc.dram_tensor("oo12", [128, NU], F16, kind="ExternalInput")
    iota_ext = nc.dram_tensor("iota_c", [128, 128], F16, kind="ExternalInput")
    maskC_ext = nc.dram_tensor("maskC", [128, nch * 2], U8, kind="ExternalInput")
    minmask_ext = nc.dram_tensor("minmask", [128, nch * 2], F32, kind="ExternalInput")
    ident_ext = nc.dram_tensor("ident_c", [128, 128], F16, kind="ExternalInput")
    W_ext = [nc.dram_tensor("W1", [2, F0, F1], F16, kind="ExternalInput"),
             nc.dram_tensor("W2", [2, F1, F2], F16, kind="ExternalInput"),
             nc.dram_tensor("W3", [2, F2, F3], F16, kind="ExternalInput")]
    L_ext = [nc.dram_tensor("loop1", [F0, F1], F16, kind="ExternalInput"),
             nc.dram_tensor("loop2", [F1, F2], F16, kind="ExternalInput"),
             nc.dram_tensor("loop3", [F2, F3], F16, kind="ExternalInput")]
    b_ext = [nc.dram_tensor("b1", [F1], F32, kind="ExternalInput"),
             nc.dram_tensor("b2", [F2], F32, kind="ExternalInput"),
             nc.dram_tensor("b3", [F3], F32, kind="ExternalInput")]
    out_ext = nc.dram_tensor("out", [128, nch * 2], F32, kind="ExternalOutput")

    table = [None,
             nc.dram_tensor("table1", [NP, 2 * F1], F16, kind="Internal",
                            addr_space="Shared"),
             nc.dram_tensor("table2", [NP, 2 * F2], F16, kind="Internal",
                            addr_space="Shared")]
    h_loc = [None,
             nc.dram_tensor("h1_loc", [NLP, F1], F16, kind="Internal"),
             nc.dram_tensor("h2_loc", [NLP, F2], F16, kind="Internal")]
    ccmin_in = nc.dram_tensor("ccmin_in", [1, 1], F32, kind="Internal")
    ccmin_out = nc.dram_tensor("ccmin_out", [cfg.n_cores, 1], F32,
                               kind="Internal", addr_space="Shared")

    F_in = [F0, F1, F2]
    F_out = [F1, F2, F3]
    rg = [list(range(cfg.n_cores))]

    with tile.TileContext(nc) as tc:
        with tc.tile_pool(name="const", bufs=1) as cp, \
             tc.tile_pool(name="hT", bufs=2) as hp, \
             tc.tile_pool(name="msg", bufs=18) as mp, \
             tc.tile_pool(name="sS", bufs=2) as sp, \
             tc.tile_pool(name="sB", bufs=2) as spB, \
             tc.tile_pool(name="aggA", bufs=9) as apA, \
             tc.tile_pool(name="tt", bufs=4) as ttp, \
             tc.tile_pool(name="pa", bufs=4, space="PSUM") as pa_pool, \
             tc.tile_pool(name="po", bufs=2, space="PSUM") as po_pool, \
             tc.tile_pool(name="ptp", bufs=2, space="PSUM") as ptp_pool:

            # ---- constants (gather-critical first) ----
            idx0_sb = cp.tile([128, NT0 * 8], I16, tag="idx0")
            nc.sync.dma_start(out=idx0_sb[:], in_=idx0_ext[:])
            idx12_sb = cp.tile([128, NT * 8], I16, tag="idx12")
            nc.sync.dma_start(out=idx12_sb[:], in_=idx12_ext[:])
            oo0_sb = cp.tile([128, NU0], F16, tag="oo0")
            nc.sync.dma_start(out=oo0_sb[:], in_=oo0_ext[:])
            oo12_sb = cp.tile([128, NU], F16, tag="oo12")
            nc.sync.dma_start(out=oo12_sb[:], in_=oo12_ext[:])
            iota_sb = cp.tile([128, 128], F16, tag="iota")
            nc.sync.dma_start(out=iota_sb[:], in_=iota_ext[:])
            ident_sb = cp.tile([128, 128], F16, tag="ident")
            nc.sync.dma_start(out=ident_sb[:], in_=ident_ext[:])

            w_sb, l_sb, b_sb = [], [], []
            for l in range(3):
                w0 = cp.tile([F_in[l], F_out[l]], F16, tag=f"w0_{l}")
                nc.sync.dma_start(out=w0[:], in_=W_ext[l][0])
                w1 = cp.tile([F_in[l], F_out[l]], F16, tag=f"w1_{l}")
                nc.sync.dma_start(out=w1[:], in_=W_ext[l][1])
                wl = cp.tile([F_in[l], F_out[l]], F16, tag=f"wl_{l}")
                nc.sync.dma_start(out=wl[:], in_=L_ext[l][:])
                w_sb.append((w0, w1))
                l_sb.append(wl)
                if l < 2:
                    bt = cp.tile([F_out[l], 1], F32, tag=f"b_{l}")
                    nc.sync.dma_start(out=bt[:], in_=b_ext[l][:, None])
                    b_sb.append(bt)
            b3_row = cp.tile([1, F3], F32, tag="b3row")
            nc.sync.dma_start(out=b3_row[:], in_=b_ext[2][None, :])
            b3_bcast = cp.tile([128, F3], F32, tag="b3b")
            nc.gpsimd.partition_broadcast(b3_bcast[:], b3_row[:])

            maskC_sb = cp.tile([128, nch * 2], U8, tag="maskC")
            nc.sync.dma_start(out=maskC_sb[:], in_=maskC_ext[:])
            minmask_sb = cp.tile([128, nch * 2], F32, tag="minmask")
            nc.sync.dma_start(out=minmask_sb[:], in_=minmask_ext[:])
            h3_sb = cp.tile([128, nch * 2], F32, tag="h3")

            xT_sb = hp.tile([F0, NLP], F16, tag="hT")
            nc.sync.dma_start(out=xT_sb[:], in_=xT_ext[:])
            h1T = hp.tile([F1, NLP], F16, tag="hT")
            h2T = hp.tile([F2, NLP], F16, tag="hT")
            hT = [xT_sb, h1T, h2T]

            rmax = cp.tile([128, 1], F32, tag="rmax")
            gq = 0

            def build_S(S_sb, oo_sb_, u_lo, u_n, col0=0):
                for s0 in range(0, u_n, SBK):
                    ln = min(SBK, u_n - s0)
                    nc.vector.tensor_tensor(
                        S_sb[:, (col0 + s0) * 128:(col0 + s0 + ln) * 128],
                        iota_sb[:, None, :].broadcast_to((128, ln, 128)),
                        oo_sb_[:, u_lo + s0:u_lo + s0 + ln, None]
                            .broadcast_to((128, ln, 128)),
                        AOT.is_equal)

            def finish_block(l, blk, aggT, prevT, nextT):
                fi, fo = F_in[l], F_out[l]
                ns = blk * 256
                if l < 2:
                    po = po_pool.tile([fo, 256], F32, tag="po")
                    nc.tensor.matmul(po[:], w_sb[l][0][:], aggT[:, 0::2],
                                     start=True, stop=False)
                    nc.tensor.matmul(po[:], w_sb[l][1][:], aggT[:, 1::2],
                                     start=False, stop=False)
                    nc.tensor.matmul(po[:], l_sb[l][:], prevT[:, ns:ns + 256],
                                     start=False, stop=True)
                    nc.scalar.activation(
                        nextT[:, ns:ns + 256], po[:],
                        mybir.ActivationFunctionType.Relu, bias=b_sb[l][:])
                    for k in range(2):
                        tp = ptp_pool.tile([128, fo], F16, tag="tp")
                        nc.tensor.transpose(
                            tp[:], nextT[:, ns + k * 128:ns + (k + 1) * 128],
                            ident_sb[0:fo, 0:fo])
                        tt = ttp.tile([128, fo], F16, tag="tt")
                        nc.scalar.activation(
                            tt[:], tp[:], mybir.ActivationFunctionType.Copy)
                        nc.sync.dma_start(
                            out=h_loc[l + 1][ns + k * 128:ns + (k + 1) * 128, :],
                            in_=tt[:])
                else:
                    for k in range(2):
                        po = po_pool.tile([128, F3], F32, tag="po")
                        nc.tensor.matmul(
                            po[:], aggT[:, k * 256:(k + 1) * 256:2],
                            w_sb[2][0][:], start=True, stop=False)
                        nc.tensor.matmul(
                            po[:], aggT[:, k * 256 + 1:(k + 1) * 256:2],
                            w_sb[2][1][:], start=False, stop=False)
                        nc.tensor.matmul(
                            po[:], prevT[:, ns + k * 128:ns + (k + 1) * 128],
                            l_sb[2][:], start=False, stop=True)
                        cn = blk * 2 + k
                        nc.vector.tensor_tensor(
                            h3_sb[:, cn * 2:(cn + 1) * 2], po[:], b3_bcast[:],
                            AOT.add)
                    cn = blk * 2
                    neg4 = cp.tile([128, 4], F32, tag="neg4")
                    nc.vector.tensor_scalar(
                        neg4[:], h3_sb[:, cn * 2:cn * 2 + 4], -1.0, None,
                        AOT.mult)
                    nc.vector.tensor_tensor(
                        neg4[:], neg4[:], minmask_sb[:, cn * 2:cn * 2 + 4],
                        AOT.subtract)
                    m1 = cp.tile([128, 1], F32, tag="m1")
                    nc.vector.tensor_reduce(
                        m1[:], neg4[:], mybir.AxisListType.X, AOT.max)
                    if blk == 0:
                        nc.vector.tensor_copy(rmax[:], m1[:])
                    else:
                        nc.vector.tensor_tensor(rmax[:], rmax[:], m1[:],
                                                AOT.max)

            # =================== layer 0 (single phase) ===================
            fi, fo = F_in[0], F_out[0]
            for blk in range(cfg.blocks):
                subs = {}
                for r in range(2):
                    for (st, ln) in plan0.calls[blk][r]:
                        m = mp.tile([128, GMAX, fi], F16, tag="msg")
                        srcap = xd_ext[:] if r == 0 else xd_ext[R0:, :]
                        nc.gpsimd.dma_gather(
                            m[:, 0:ln, :], srcap,
                            idx0_sb[:, st * 8:(st + ln) * 8],
                            ln * 128, ln * 128, fi, elem_step=fi,
                            queue_num=gq % NQ)
                        gq += 1
                        for t in range(ln):
                            subs[st + t] = (m, t)
                us, _, ue = plan0.ublk[blk]
                S_sb = sp.tile([128, (ue - us) * 128], F16, tag="S")
                build_S(S_sb, oo0_sb, us, ue - us)
                aggT = apA.tile([fi, 512], F16, tag="aggT")
                for c4 in range(4):
                    c = blk * 4 + c4
                    ulist = (plan0.units_cr.get((c, 0), [])
                             + plan0.units_cr.get((c, 1), []))
                    if not ulist:
                        nc.vector.memset(aggT[:, c4 * 128:(c4 + 1) * 128], 0)
                        continue
                    pa = pa_pool.tile([fi, 128], F32, tag="pa")
                    for i, (gt, p, u) in enumerate(ulist):
                        m, t = subs[gt]
                        nc.tensor.matmul(
                            pa[:], m[:, t, :],
                            S_sb[:, (u - us) * 128:(u - us + 1) * 128],
                            start=(i == 0), stop=(i == len(ulist) - 1))
                    nc.scalar.activation(aggT[:, c4 * 128:(c4 + 1) * 128],
                                         pa[:],
                                         mybir.ActivationFunctionType.Copy)
                finish_block(0, blk, aggT, hT[0], hT[1])
                if blk == HA_BLOCKS + 1:
                    nc.gpsimd.collective_compute(
                        "AllGather", AOT.bypass, replica_groups=rg,
                        ins=[h_loc[1][0:HA, :].opt()],
                        outs=[table[1][0:rowsA, :].opt()])

            # ==== layers 1/2: interleaved region phases (B lags A by K) ====
            # The B-half AllGather for table[l] is issued a little way INTO
            # layer l's A-gather stream (Pool is in-order: issuing it right
            # after the previous layer would stall the A-calls on the store
            # tail); it completes well before the first B-call at it == K.
            K = 4
            for l in (1, 2):
                fi, fo = F_in[l], F_out[l]
                prevT, nextT = hT[l], (hT[l + 1] if l < 2 else None)
                aggTs = {}
                for it in range(cfg.blocks + K):
                    if it < cfg.blocks:
                        # region-A gathers + partial aggregation for block it
                        blk = it
                        subs = {}
                        for (st, ln) in plan12.calls[blk][0]:
                            m = mp.tile([128, GMAX, 2 * fi], F16, tag="msg")
                            nc.gpsimd.dma_gather(
                                m[:, 0:ln, :], table[l][0:rowsA, :],
                                idx12_sb[:, st * 8:(st + ln) * 8],
                                ln * 128, ln * 128, 2 * fi, elem_step=2 * fi,
                                queue_num=gq % NQ)
                            gq += 1
                            for t in range(ln):
                                subs[st + t] = (m, t)
                        us, mid, ue = plan12.ublk[blk]
                        S_sb = sp.tile([128, (mid - us) * 128], F16, tag="S")
                        build_S(S_sb, oo12_sb, us, mid - us)
                        aggT = apA.tile([fi, 512], F16, tag="aggT")
                        for c4 in range(4):
                            c = blk * 4 + c4
                            ulist = plan12.units_cr.get((c, 0), [])
                            if not ulist:
                                nc.vector.memset(
                                    aggT[:, c4 * 128:(c4 + 1) * 128], 0)
                                continue
                            pa = pa_pool.tile([fi, 128], F32, tag="pa")
                            for i, (gt, p, u) in enumerate(ulist):
                                m, t = subs[gt]
                                nc.tensor.matmul(
                                    pa[:], m[:, t, p * fi:(p + 1) * fi],
                                    S_sb[:, (u - us) * 128:(u - us + 1) * 128],
                                    start=(i == 0), stop=(i == len(ulist) - 1))
                            nc.scalar.activation(
                                aggT[:, c4 * 128:(c4 + 1) * 128], pa[:],
                                mybir.ActivationFunctionType.Copy)
                        aggTs[blk] = aggT
                    if it == 1:
                        # B-half table for THIS layer's region-1 gathers
                        nc.gpsimd.collective_compute(
                            "AllGather", AOT.bypass, replica_groups=rg,
                            ins=[h_loc[l][HA:NL, :].opt()],
                            outs=[table[l][rowsA:NP, :].opt()])
                    if it >= K:
                        # region-B gathers + finalize block it-K
                        blk = it - K
                        subs = {}
                        for (st, ln) in plan12.calls[blk][1]:
                            m = mp.tile([128, GMAX, 2 * fi], F16, tag="msg")
                            nc.gpsimd.dma_gather(
                                m[:, 0:ln, :], table[l][rowsA:, :],
                                idx12_sb[:, st * 8:(st + ln) * 8],
                                ln * 128, ln * 128, 2 * fi, elem_step=2 * fi,
                                queue_num=gq % NQ)
                            gq += 1
                            for t in range(ln):
                                subs[st + t] = (m, t)
                        us, mid, ue = plan12.ublk[blk]
                        aggT = aggTs.pop(blk)
                        if ue > mid:
                            S_sb = spB.tile([128, (ue - mid) * 128], F16,
                                            tag="SB")
                            build_S(S_sb, oo12_sb, mid, ue - mid)
                            for c4 in range(4):
                                c = blk * 4 + c4
                                ulist = plan12.units_cr.get((c, 1), [])
                                if not ulist:
                                    continue
                                pa = pa_pool.tile([fi, 128], F32, tag="pa")
                                # seed the accumulator with the region-A
                                # partial via an identity matmul, then add
                                # the region-B units; one Copy writes back.
                                nc.tensor.matmul(
                                    pa[:], ident_sb[0:fi, 0:fi],
                                    aggT[:, c4 * 128:(c4 + 1) * 128],
                                    start=True, stop=False)
                                for i, (gt, p, u) in enumerate(ulist):
                                    m, t = subs[gt]
                                    nc.tensor.matmul(
                                        pa[:], m[:, t, p * fi:(p + 1) * fi],
                                        S_sb[:, (u - mid) * 128:
                                             (u - mid + 1) * 128],
                                        start=False,
                                        stop=(i == len(ulist) - 1))
                                nc.scalar.activation(
                                    aggT[:, c4 * 128:(c4 + 1) * 128], pa[:],
                                    mybir.ActivationFunctionType.Copy)
                        finish_block(l, blk, aggT, prevT, nextT)
                        if l < 2 and blk == HA_BLOCKS + 1:
                            nc.gpsimd.collective_compute(
                                "AllGather", AOT.bypass, replica_groups=rg,
                                ins=[h_loc[l + 1][0:HA, :].opt()],
                                outs=[table[l + 1][0:rowsA, :].opt()])
                if l < 2:
                    nc.gpsimd.collective_compute(
                        "AllGather", AOT.bypass, replica_groups=rg,
                        ins=[h_loc[l + 1][HA:NL, :].opt()],
                        outs=[table[l + 1][rowsA:NP, :].opt()])

            # ---- global min (via negate+max) + action mask ----
            mar = cp.tile([128, 1], F32, tag="mar")
            nc.gpsimd.partition_all_reduce(mar[:], rmax[:], 128,
                                           bass_isa.ReduceOp.max)
            nc.sync.dma_start(out=ccmin_in[:], in_=mar[0:1, :])
            nc.gpsimd.collective_compute(
                "AllGather", AOT.bypass, replica_groups=rg,
                ins=[ccmin_in[:].opt()], outs=[ccmin_out[:].opt()])
            gmaxs = cp.tile([1, cfg.n_cores], F32, tag="gmaxs")
            nc.sync.dma_start(out=gmaxs[:], in_=ccmin_out[:, 0][None, :])
            gmax = cp.tile([1, 1], F32, tag="gmax")
            nc.vector.tensor_reduce(gmax[:], gmaxs[:], mybir.AxisListType.X, AOT.max)
            gm1 = cp.tile([1, 1], F32, tag="gm1")
            nc.vector.tensor_scalar(gm1[:], gmax[:], -1.0, -1.0, AOT.mult, AOT.add)
            gm1b = cp.tile([128, 1], F32, tag="gm1b")
            nc.gpsimd.partition_broadcast(gm1b[:], gm1[:])
            repl = cp.tile([128, nch * 2], F32, tag="repl")
            nc.vector.tensor_scalar(repl[:], h3_sb[:], 0.0, gm1b[:],
                                    AOT.mult, AOT.add)
            nc.vector.copy_predicated(h3_sb[:], maskC_sb[:], repl[:])
            nc.sync.dma_start(out=out_ext[:], in_=h3_sb[:])

    nc.compile()
    return nc


def run(cfg, inputs, trace=False):
    x = np.asarray(inputs["x"], dtype=np.float32)
    src = np.asarray(inputs["src"]).astype(np.int64)
    dst = np.asarray(inputs["dst"]).astype(np.int64)
    et = np.asarray(inputs["etypes"]).astype(np.int64)
    cs = np.asarray(inputs["cell_size"]).astype(np.int64)
    ms = np.asarray(inputs["max_size"]).astype(np.int64)

    (plan0, plan12, idx0_arrs, oo0_arrs, idx12_arrs, oo12_arrs,
     xT, maskC, minmask) = preprocess(cfg, x, src, dst, et, cs, ms)
    nc = build_program(cfg, plan0, plan12)

    iota_c = np.broadcast_to(np.arange(128, dtype=np.float16), (128, 128)).copy()
    ident_c = np.eye(128, dtype=np.float16)
    common = dict(
        xdir=x.astype(np.float16), ident_c=ident_c, iota_c=iota_c,
        W1=np.asarray(inputs["W1"], np.float16),
        loop1=np.asarray(inputs["loop1"], np.float16),
        b1=np.asarray(inputs["b1"], np.float32),
        W2=np.asarray(inputs["W2"], np.float16),
        loop2=np.asarray(inputs["loop2"], np.float16),
        b2=np.asarray(inputs["b2"], np.float32),
        W3=np.asarray(inputs["W3"], np.float16),
        loop3=np.asarray(inputs["loop3"], np.float16),
        b3=np.asarray(inputs["b3"], np.float32),
    )
    in_maps = []
    for c in range(cfg.n_cores):
        m = dict(common)
        m["xT"] = xT[c]
        m["idx0"] = idx0_arrs[c]
        m["oo0"] = oo0_arrs[c]
        m["idx12"] = idx12_arrs[c]
        m["oo12"] = oo12_arrs[c]
        m["maskC"] = maskC[c]
        m["minmask"] = minmask[c]
        in_maps.append(m)

    import os as _os
    tmpdir = _os.environ.get("GNN_TRACE_DIR") or None
    nch = cfg.NLP // 128
    for attempt in range(3):
        res = run_bass_kernel_spmd(nc, in_maps, list(range(cfg.n_cores)),
                                   trace=trace, tmpdir=tmpdir)
        out = np.empty((cfg.N, 2), dtype=np.float32)
        for c in range(cfg.n_cores):
            o = res.results[c]["out"]
            o = o.reshape(128, nch, 2).transpose(1, 0, 2).reshape(cfg.NLP, 2)
            out[c * cfg.NL:(c + 1) * cfg.NL] = o[:cfg.NL]
        if np.isfinite(out).all():
            break
    return out, res


def kernel(**inputs):
    cfg = Cfg(N=50000, E=800000, feats=[128, 64, 64, 2], n_cores=8)
    out, _ = run(cfg, inputs)
    return out


# revision 25
# speedup vs baseline: 1.0023x; 1.0023x over previous
"""RelGraphConv (3-layer, 2-relation) GNN message passing on 8 trn2 NeuronCores.

Strategy: partition nodes across cores (graph parallel). Per layer, each core
gathers raw source-node features for its incoming edges (dma_gather from a
replicated HBM feature table), aggregates per (dst, relation) slot with
one-hot matmuls accumulated in PSUM, applies the per-relation weights after
aggregation (the conv is linear, so W can be applied post-aggregation), and
AllGathers the new node features into the next layer's table.

Implementation notes (hardware-driven):
- The DMA engines cost ~constant time per gather descriptor up to ~512B.
  Layer 0 gathers direct 256B rows (x is [N,128] fp16) with a two-range src
  split (dma_gather idx is int16: rows >= 32768 gather from a base-offset
  view). Layers 1/2 (64 feats = 128B < the 256B DMA minimum) keep
  pair-packed 256B rows with parity-masked one-hot S matmuls.
- The inter-layer AllGather is split at HA nodes per core, and layer l+1's
  edge stream is split per block into region-A tiles (src in the first
  AllGather half) and region-B tiles. All A gathers of the whole layer run
  as phase A (they only need the early collective), then phase B gathers
  the rest and finishes each block: this removes the Pool idle bubble at
  layer boundaries (the collectives' DMA traffic overlaps phase A/B).
- Tiles are padded per (block, region); units are (tile, chunk, parity)
  runs taken as a union over cores, masked per-core via oo=255 columns.
- dma_gather wedges above 1024 indices per call -> sub-gathers of <= 8
  tiles round-robined over 4 SWDGE queues. S matrices are built 8 tiles
  per DVE op via step-0 broadcast APs. fp16 feature path; PSUM fp32.
"""
import sys

sys.path.insert(0, "/opt/trn_rl_repo")

import numpy as np

import concourse.bacc as bacc
import concourse.bass as bass
import concourse.bass_isa as bass_isa
import concourse.tile as tile
from concourse import mybir
from concourse.bass_utils import run_bass_kernel_spmd

F32 = mybir.dt.float32
F16 = mybir.dt.float16
I16 = mybir.dt.int16
U8 = mybir.dt.uint8
AOT = mybir.AluOpType

GMAX = 8   # tiles per dma_gather (1024 idx hardware limit)
SBK = 8    # tiles per batched S-build
NQ = 4     # SWDGE queues
HA_BLOCKS = 19  # blocks per core in the first AllGather half
R0 = 32768      # L0 direct-gather low range size (int16 idx limit)


class Cfg:
    def __init__(self, N, E, feats, n_cores=8):
        self.N = N
        self.E = E
        self.feats = feats          # [F0, F1, F2, F3]
        self.n_cores = n_cores
        self.NL = N // n_cores      # nodes per core (must divide)
        assert self.NL * n_cores == N
        assert N % 2 == 0 and N // 2 < 32768
        # pad local nodes to blocks of 256 (= 4 chunks of 128 slots)
        self.NLP = ((self.NL + 255) // 256) * 256
        self.blocks = self.NLP // 256
        self.chunks = self.blocks * 4
        self.HA = min(HA_BLOCKS * 256, self.NL)   # first-half node count
        assert self.HA % 2 == 0 and (self.NL - self.HA) % 2 == 0


class SplitPlan:
    """Edge-stream plan with a per-(block, region) tile split.

    Regions: 0 = first table region (or low src range for L0), 1 = rest.
    Per block: region-0 tiles (edges sorted by (chunk, parity, ...)) then
    region-1 tiles. One matmul unit per (tile, chunk, parity) run present on
    any core. Calls are per (block, region) in <= GMAX-tile pieces.
    """

    def __init__(self, cfg, edges_key, n_cores):
        # edges_key: (core, blk, region, chunk4, parity, eidx) rows [E, 6]
        self.cfg = cfg
        E = len(edges_key)
        core, blk, regn, ch4, par = (edges_key[:, k] for k in range(5))
        blocks, chunks = cfg.blocks, cfg.chunks
        # counts per (core, blk, region)
        cbr = (core * blocks + blk) * 2 + regn
        cnt_cbr = np.bincount(cbr, minlength=n_cores * blocks * 2).reshape(
            n_cores, blocks, 2)
        cap = np.ceil(cnt_cbr.max(axis=0) / 128).astype(np.int64)  # [blk, 2]
        cap[(cap.sum(axis=1) == 0), 0] = 1
        self.cap = cap
        # global tile offsets: block-major, region 0 tiles then region 1
        self.t_off = np.zeros((blocks, 2), dtype=np.int64)
        pos = 0
        for b in range(blocks):
            self.t_off[b, 0] = pos
            pos += cap[b, 0]
            self.t_off[b, 1] = pos
            pos += cap[b, 1]
        self.n_tiles = pos

        # per-core edge position within its (core, blk, region) stream,
        # ordered by (chunk4, parity, eidx)
        order = np.lexsort((edges_key[:, 5], par, ch4, regn, blk, core))
        base_cbr = np.zeros(n_cores * blocks * 2, dtype=np.int64)
        np.cumsum(cnt_cbr.reshape(-1)[:-1], out=base_cbr[1:])
        pos_in = np.arange(E) - base_cbr[cbr[order]]
        gt = self.t_off[blk[order], regn[order]] + pos_in // 128
        self.order = order
        self.gtile = gt
        self.slot_pp = pos_in % 128

        # units: (tile, parity) pairs present on any core, tile-major.
        # A tile's rows may span chunks; the chunk is encoded per unit for
        # the matmul's pa target. unit key = (gt, chunk, parity).
        ukeys = np.unique(
            np.stack([gt, ch4[order] + 4 * blk[order], par[order]], axis=1),
            axis=0)
        # order units by (tile, chunk, parity) -> contiguous per block
        uord = np.lexsort((ukeys[:, 2], ukeys[:, 1], ukeys[:, 0]))
        ukeys = ukeys[uord]
        self.n_units = len(ukeys)
        self.umap = {tuple(k): i for i, k in enumerate(ukeys)}
        # per (chunk, region): ordered list of (gt, parity, unit)
        self.units_cr = {}
        for i, (g_, c_, p_) in enumerate(ukeys):
            r_ = 1 if g_ >= self.t_off[c_ // 4, 1] else 0
            self.units_cr.setdefault((int(c_), r_), []).append(
                (int(g_), int(p_), i))
        # unit range per block (for S builds): units are tile-major and tiles
        # are block-major, so each (block, region) owns a contiguous range.
        self.ublk = []
        for b in range(blocks):
            lo = self.t_off[b, 0]
            hi = self.t_off[b, 1] + cap[b, 1]
            us = np.searchsorted(ukeys[:, 0], lo, side="left")
            mid = np.searchsorted(ukeys[:, 0], self.t_off[b, 1], side="left")
            ue = np.searchsorted(ukeys[:, 0], hi, side="left")
            self.ublk.append((int(us), int(mid), int(ue)))
        # calls per (block, region)
        self.calls = []
        for b in range(blocks):
            cl = [[], []]
            for r in range(2):
                n = cap[b, r]
                st = self.t_off[b, r]
                for s in range(0, n, GMAX):
                    cl[r].append((int(st + s), int(min(GMAX, n - s))))
            self.calls.append(cl)

    def u_of_edges(self, blk_o, ch4_o, par_o):
        gt = self.gtile
        keys = np.stack([gt, ch4_o + 4 * blk_o, par_o], axis=1)
        return np.array([self.umap[tuple(k)] for k in keys], dtype=np.int64)


def preprocess(cfg, x, src, dst, etypes, cell_size, max_size):
    n_cores, NL, NLP, HA = cfg.n_cores, cfg.NL, cfg.NLP, cfg.HA
    HB = NL - HA
    E = len(src)
    core_of = dst // NL
    o = 2 * (dst - core_of * NL) + etypes
    blk = o // 512
    ch4 = (o // 128) % 4
    oo = (o % 128).astype(np.int64)

    # ---------- L0: direct rows, regions = src ranges, no parity ----------
    rng0 = (src >= R0).astype(np.int64)
    ek0 = np.stack([core_of, blk, rng0, ch4,
                    np.zeros(E, dtype=np.int64), np.arange(E)], axis=1)
    plan0 = SplitPlan(cfg, ek0, n_cores)
    idxv0 = np.where(rng0 == 0, src, src - R0).astype(np.int16)
    o0 = plan0.order
    u0 = plan0.u_of_edges(blk[o0], ch4[o0], np.zeros(E, dtype=np.int64))

    NI0 = plan0.n_tiles * 128
    NU0 = plan0.n_units
    idx0_arrs, oo0_arrs = [], []
    for c in range(n_cores):
        sel = core_of[o0] == c
        ia = np.zeros(NI0, dtype=np.int16)
        ia[plan0.gtile[sel] * 128 + plan0.slot_pp[sel]] = idxv0[o0][sel]
        idx0_arrs.append(np.tile(ia.reshape(NI0 // 16, 16).T, (8, 1)))
        ou = np.full((128, NU0), 255.0, dtype=np.float16)
        ou[plan0.slot_pp[sel], u0[sel]] = oo[o0][sel].astype(np.float16)
        oo0_arrs.append(ou)

    # ---------- L1/L2: pair rows, regions = AllGather halves ----------
    score = src // NL
    soff = src - score * NL
    g = np.where(soff < HA,
                 score * HA + soff,
                 n_cores * HA + score * HB + (soff - HA))
    idxv12 = (g >> 1).astype(np.int16)
    par = (src & 1).astype(np.int64)
    regn = (soff >= HA).astype(np.int64)
    rowsA = n_cores * HA // 2
    # region-1 gathers use a base offset of rowsA pair rows
    idxv12 = np.where(regn == 0, idxv12, idxv12 - rowsA).astype(np.int16)

    ek = np.stack([core_of, blk, regn, ch4, par, np.arange(E)], axis=1)
    plan12 = SplitPlan(cfg, ek, n_cores)
    o12 = plan12.order
    u12 = plan12.u_of_edges(blk[o12], ch4[o12], par[o12])

    NI = plan12.n_tiles * 128
    NU = plan12.n_units
    idx12_arrs, oo12_arrs = [], []
    for c in range(n_cores):
        sel = core_of[o12] == c
        ia = np.zeros(NI, dtype=np.int16)
        ia[plan12.gtile[sel] * 128 + plan12.slot_pp[sel]] = idxv12[o12][sel]
        idx12_arrs.append(np.tile(ia.reshape(NI // 16, 16).T, (8, 1)))
        ou = np.full((128, NU), 255.0, dtype=np.float16)
        ou[plan12.slot_pp[sel], u12[sel]] = oo[o12][sel].astype(np.float16)
        oo12_arrs.append(ou)

    xT, maskC, minmask = [], [], []
    for c in range(n_cores):
        xl = x[c * NL:(c + 1) * NL]
        xt = np.zeros((cfg.feats[0], NLP), dtype=np.float16)
        xt[:, :NL] = xl.T.astype(np.float16)
        xT.append(xt)
        cs = cell_size[c * NL:(c + 1) * NL]
        ms = max_size[c * NL:(c + 1) * NL]
        m = np.zeros((NLP, 2), dtype=np.float32)
        m[:NL, 0] = cs >= (ms - 1)
        m[:NL, 1] = cs == 0
        mm = np.zeros((NLP, 2), dtype=np.float32)
        mm[NL:, :] = 1e30
        nch = NLP // 128
        maskC.append(m.reshape(nch, 128, 2).transpose(1, 0, 2)
                     .reshape(128, nch * 2).astype(np.uint8))
        minmask.append(mm.reshape(nch, 128, 2).transpose(1, 0, 2)
                       .reshape(128, nch * 2).copy())

    return plan0, plan12, idx0_arrs, oo0_arrs, idx12_arrs, oo12_arrs, \
        xT, maskC, minmask


def build_program(cfg, plan0, plan12):
    F0, F1, F2, F3 = cfg.feats
    NLP, NL, HA = cfg.NLP, cfg.NL, cfg.HA
    NT0, NT = plan0.n_tiles, plan12.n_tiles
    NP = cfg.N // 2
    nch = NLP // 128
    n_cores = cfg.n_cores
    rowsA = n_cores * HA // 2

    nc = bacc.Bacc(None, target_bir_lowering=False, debug=False,
                   num_devices=cfg.n_cores, num_swdge_queues=NQ,
                   dynamic_dma_scratch_size=32768)

    xd_ext = nc.dram_tensor("xdir", [cfg.N, F0], F16, kind="ExternalInput")
    xT_ext = nc.dram_tensor("xT", [F0, NLP], F16, kind="ExternalInput")
    idx0_ext = nc.dram_tensor("idx0", [128, NT0 * 8], I16, kind="ExternalInput")
    idx12_ext = nc.dram_tensor("idx12", [128, NT * 8], I16, kind="ExternalInput")
    NU0, NU = plan0.n_units, plan12.n_units
    oo0_ext = nc.dram_tensor("oo0", [128, NU0], F16, kind="ExternalInput")
    oo12_ext = nc.dram_tensor("oo12", [128, NU], F16, kind="ExternalInput")
    iota_ext = nc.dram_tensor("iota_c", [128, 128], F16, kind="ExternalInput")
    maskC_ext = nc.dram_tensor("maskC", [128, nch * 2], U8, kind="ExternalInput")
    minmask_ext = nc.dram_tensor("minmask", [128, nch * 2], F32, kind="ExternalInput")
    ident_ext = nc.dram_tensor("ident_c", [128, 128], F16, kind="ExternalInput")
    W_ext = [nc.dram_tensor("W1", [2, F0, F1], F16, kind="ExternalInput"),
             nc.dram_tensor("W2", [2, F1, F2], F16, kind="ExternalInput"),
             nc.dram_tensor("W3", [2, F2, F3], F16, kind="ExternalInput")]
    L_ext = [nc.dram_tensor("loop1", [F0, F1], F16, kind="ExternalInput"),
             nc.dram_tensor("loop2", [F1, F2], F16, kind="ExternalInput"),
             nc.dram_tensor("loop3", [F2, F3], F16, kind="ExternalInput")]
    b_ext = [nc.dram_tensor("b1", [F1], F32, kind="ExternalInput"),
             nc.dram_tensor("b2", [F2], F32, kind="ExternalInput"),
             nc.dram_tensor("b3", [F3], F32, kind="ExternalInput")]
    out_ext = nc.dram_tensor("out", [128, nch * 2], F32, kind="ExternalOutput")

    table = [None,
             nc.dram_tensor("table1", [NP, 2 * F1], F16, kind="Internal",
                            addr_space="Shared"),
             nc.dram_tensor("table2", [NP, 2 * F2], F16, kind="Internal",
                            addr_space="Shared")]
    h_loc = [None,
             nc.dram_tensor("h1_loc", [NLP, F1], F16, kind="Internal"),
             nc.dram_tensor("h2_loc", [NLP, F2], F16, kind="Internal")]
    ccmin_in = nc.dram_tensor("ccmin_in", [1, 1], F32, kind="Internal")
    ccmin_out = nc.dram_tensor("ccmin_out", [cfg.n_cores, 1], F32,
                               kind="Internal", addr_space="Shared")

    F_in = [F0, F1, F2]
    F_out = [F1, F2, F3]
    rg = [list(range(cfg.n_cores))]

    with tile.TileContext(nc) as tc:
        with tc.tile_pool(name="const", bufs=1) as cp, \
             tc.tile_pool(name="hT", bufs=2) as hp, \
             tc.tile_pool(name="msg", bufs=18) as mp, \
             tc.tile_pool(name="sS", bufs=2) as sp, \
             tc.tile_pool(name="sB", bufs=2) as spB, \
             tc.tile_pool(name="aggA", bufs=9) as apA, \
             tc.tile_pool(name="tt", bufs=4) as ttp, \
             tc.tile_pool(name="pa", bufs=4, space="PSUM") as pa_pool, \
             tc.tile_pool(name="po", bufs=2, space="PSUM") as po_pool, \
             tc.tile_pool(name="ptp", bufs=2, space="PSUM") as ptp_pool:

            # ---- constants (gather-critical first) ----
            idx0_sb = cp.tile([128, NT0 * 8], I16, tag="idx0")
            ih = (NT0 * 8) // 4
            nc.sync.dma_start(out=idx0_sb[:, 0:ih], in_=idx0_ext[:, 0:ih])
            nc.sync.dma_start(out=idx0_sb[:, ih:], in_=idx0_ext[:, ih:])
            idx12_sb = cp.tile([128, NT * 8], I16, tag="idx12")
            nc.sync.dma_start(out=idx12_sb[:], in_=idx12_ext[:])
            oo0_sb = cp.tile([128, NU0], F16, tag="oo0")
            nc.sync.dma_start(out=oo0_sb[:], in_=oo0_ext[:])
            oo12_sb = cp.tile([128, NU], F16, tag="oo12")
            nc.sync.dma_start(out=oo12_sb[:], in_=oo12_ext[:])
            iota_sb = cp.tile([128, 128], F16, tag="iota")
            nc.sync.dma_start(out=iota_sb[:], in_=iota_ext[:])
            ident_sb = cp.tile([128, 128], F16, tag="ident")
            nc.sync.dma_start(out=ident_sb[:], in_=ident_ext[:])

            w_sb, l_sb, b_sb = [], [], []
            for l in range(3):
                w0 = cp.tile([F_in[l], F_out[l]], F16, tag=f"w0_{l}")
                nc.sync.dma_start(out=w0[:], in_=W_ext[l][0])
                w1 = cp.tile([F_in[l], F_out[l]], F16, tag=f"w1_{l}")
                nc.sync.dma_start(out=w1[:], in_=W_ext[l][1])
                wl = cp.tile([F_in[l], F_out[l]], F16, tag=f"wl_{l}")
                nc.sync.dma_start(out=wl[:], in_=L_ext[l][:])
                w_sb.append((w0, w1))
                l_sb.append(wl)
                if l < 2:
                    bt = cp.tile([F_out[l], 1], F32, tag=f"b_{l}")
                    nc.sync.dma_start(out=bt[:], in_=b_ext[l][:, None])
                    b_sb.append(bt)
            b3_row = cp.tile([1, F3], F32, tag="b3row")
            nc.sync.dma_start(out=b3_row[:], in_=b_ext[2][None, :])
            b3_bcast = cp.tile([128, F3], F32, tag="b3b")
            nc.gpsimd.partition_broadcast(b3_bcast[:], b3_row[:])

            maskC_sb = cp.tile([128, nch * 2], U8, tag="maskC")
            nc.sync.dma_start(out=maskC_sb[:], in_=maskC_ext[:])
            minmask_sb = cp.tile([128, nch * 2], F32, tag="minmask")
            nc.sync.dma_start(out=minmask_sb[:], in_=minmask_ext[:])
            h3_sb = cp.tile([128, nch * 2], F32, tag="h3")

            xT_sb = hp.tile([F0, NLP], F16, tag="hT")
            nc.sync.dma_start(out=xT_sb[:], in_=xT_ext[:])
            h1T = hp.tile([F1, NLP], F16, tag="hT")
            h2T = hp.tile([F2, NLP], F16, tag="hT")
            hT = [xT_sb, h1T, h2T]

            rmax = cp.tile([128, 1], F32, tag="rmax")
            gq = 0

            def build_S(S_sb, oo_sb_, u_lo, u_n, col0=0):
                for s0 in range(0, u_n, SBK):
                    ln = min(SBK, u_n - s0)
                    nc.vector.tensor_tensor(
                        S_sb[:, (col0 + s0) * 128:(col0 + s0 + ln) * 128],
                        iota_sb[:, None, :].broadcast_to((128, ln, 128)),
                        oo_sb_[:, u_lo + s0:u_lo + s0 + ln, None]
                            .broadcast_to((128, ln, 128)),
                        AOT.is_equal)

            def finish_block(l, blk, aggT, prevT, nextT):
                fi, fo = F_in[l], F_out[l]
                ns = blk * 256
                if l < 2:
                    po = po_pool.tile([fo, 256], F32, tag="po")
                    nc.tensor.matmul(po[:], w_sb[l][0][:], aggT[:, 0::2],
                                     start=True, stop=False)
                    nc.tensor.matmul(po[:], w_sb[l][1][:], aggT[:, 1::2],
                                     start=False, stop=False)
                    nc.tensor.matmul(po[:], l_sb[l][:], prevT[:, ns:ns + 256],
                                     start=False, stop=True)
                    nc.scalar.activation(
                        nextT[:, ns:ns + 256], po[:],
                        mybir.ActivationFunctionType.Relu, bias=b_sb[l][:])
                    for k in range(2):
                        tp = ptp_pool.tile([128, fo], F16, tag="tp")
                        nc.tensor.transpose(
                            tp[:], nextT[:, ns + k * 128:ns + (k + 1) * 128],
                            ident_sb[0:fo, 0:fo])
                        tt = ttp.tile([128, fo], F16, tag="tt")
                        nc.scalar.activation(
                            tt[:], tp[:], mybir.ActivationFunctionType.Copy)
                        nc.sync.dma_start(
                            out=h_loc[l + 1][ns + k * 128:ns + (k + 1) * 128, :],
                            in_=tt[:])
                else:
                    for k in range(2):
                        po = po_pool.tile([128, F3], F32, tag="po")
                        nc.tensor.matmul(
                            po[:], aggT[:, k * 256:(k + 1) * 256:2],
                            w_sb[2][0][:], start=True, stop=False)
                        nc.tensor.matmul(
                            po[:], aggT[:, k * 256 + 1:(k + 1) * 256:2],
                            w_sb[2][1][:], start=False, stop=False)
                        nc.tensor.matmul(
                            po[:], prevT[:, ns + k * 128:ns + (k + 1) * 128],
                            l_sb[2][:], start=False, stop=True)
                        cn = blk * 2 + k
                        nc.vector.tensor_tensor(
                            h3_sb[:, cn * 2:(cn + 1) * 2], po[:], b3_bcast[:],
                            AOT.add)
                    cn = blk * 2
                    neg4 = cp.tile([128, 4], F32, tag="neg4")
                    nc.vector.tensor_scalar(
                        neg4[:], h3_sb[:, cn * 2:cn * 2 + 4], -1.0, None,
                        AOT.mult)
                    nc.vector.tensor_tensor(
                        neg4[:], neg4[:], minmask_sb[:, cn * 2:cn * 2 + 4],
                        AOT.subtract)
                    m1 = cp.tile([128, 1], F32, tag="m1")
                    nc.vector.tensor_reduce(
                        m1[:], neg4[:], mybir.AxisListType.X, AOT.max)
                    if blk == 0:
                        nc.vector.tensor_copy(rmax[:], m1[:])
                    else:
                        nc.vector.tensor_tensor(rmax[:], rmax[:], m1[:],
                                                AOT.max)

            # =================== layer 0 (single phase) ===================
            fi, fo = F_in[0], F_out[0]
            for blk in range(cfg.blocks):
                subs = {}
                for r in range(2):
                    for (st, ln) in plan0.calls[blk][r]:
                        m = mp.tile([128, GMAX, fi], F16, tag="msg")
                        srcap = xd_ext[:] if r == 0 else xd_ext[R0:, :]
                        nc.gpsimd.dma_gather(
                            m[:, 0:ln, :], srcap,
                            idx0_sb[:, st * 8:(st + ln) * 8],
                            ln * 128, ln * 128, fi, elem_step=fi,
                            queue_num=gq % NQ)
                        gq += 1
                        for t in range(ln):
                            subs[st + t] = (m, t)
                us, _, ue = plan0.ublk[blk]
                S_sb = sp.tile([128, (ue - us) * 128], F16, tag="S")
                build_S(S_sb, oo0_sb, us, ue - us)
                aggT = apA.tile([fi, 512], F16, tag="aggT")
                for c4 in range(4):
                    c = blk * 4 + c4
                    ulist = (plan0.units_cr.get((c, 0), [])
                             + plan0.units_cr.get((c, 1), []))
                    if not ulist:
                        nc.vector.memset(aggT[:, c4 * 128:(c4 + 1) * 128], 0)
                        continue
                    pa = pa_pool.tile([fi, 128], F32, tag="pa")
                    for i, (gt, p, u) in enumerate(ulist):
                        m, t = subs[gt]
                        nc.tensor.matmul(
                            pa[:], m[:, t, :],
                            S_sb[:, (u - us) * 128:(u - us + 1) * 128],
                            start=(i == 0), stop=(i == len(ulist) - 1))
                    nc.scalar.activation(aggT[:, c4 * 128:(c4 + 1) * 128],
                                         pa[:],
                                         mybir.ActivationFunctionType.Copy)
                finish_block(0, blk, aggT, hT[0], hT[1])
                if blk == HA_BLOCKS + 1:
                    nc.gpsimd.collective_compute(
                        "AllGather", AOT.bypass, replica_groups=rg,
                        ins=[h_loc[1][0:HA, :].opt()],
                        outs=[table[1][0:rowsA, :].opt()])

            # ==== layers 1/2: interleaved region phases (B lags A by K) ====
            # The B-half AllGather for table[l] is issued a little way INTO
            # layer l's A-gather stream (Pool is in-order: issuing it right
            # after the previous layer would stall the A-calls on the store
            # tail); it completes well before the first B-call at it == K.
            K = 4
            for l in (1, 2):
                fi, fo = F_in[l], F_out[l]
                prevT, nextT = hT[l], (hT[l + 1] if l < 2 else None)
                aggTs = {}
                for it in range(cfg.blocks + K):
                    if it < cfg.blocks:
                        # region-A gathers + partial aggregation for block it
                        blk = it
                        subs = {}
                        for (st, ln) in plan12.calls[blk][0]:
                            m = mp.tile([128, GMAX, 2 * fi], F16, tag="msg")
                            nc.gpsimd.dma_gather(
                                m[:, 0:ln, :], table[l][0:rowsA, :],
                                idx12_sb[:, st * 8:(st + ln) * 8],
                                ln * 128, ln * 128, 2 * fi, elem_step=2 * fi,
                                queue_num=gq % NQ)
                            gq += 1
                            for t in range(ln):
                                subs[st + t] = (m, t)
                        us, mid, ue = plan12.ublk[blk]
                        S_sb = sp.tile([128, (mid - us) * 128], F16, tag="S")
                        build_S(S_sb, oo12_sb, us, mid - us)
                        aggT = apA.tile([fi, 512], F16, tag="aggT")
                        for c4 in range(4):
                            c = blk * 4 + c4
                            ulist = plan12.units_cr.get((c, 0), [])
                            if not ulist:
                                nc.vector.memset(
                                    aggT[:, c4 * 128:(c4 + 1) * 128], 0)
                                continue
                            pa = pa_pool.tile([fi, 128], F32, tag="pa")
                            for i, (gt, p, u) in enumerate(ulist):
                                m, t = subs[gt]
                                nc.tensor.matmul(
                                    pa[:], m[:, t, p * fi:(p + 1) * fi],
                                    S_sb[:, (u - us) * 128:(u - us + 1) * 128],
                                    start=(i == 0), stop=(i == len(ulist) - 1))
                            nc.scalar.activation(
                                aggT[:, c4 * 128:(c4 + 1) * 128], pa[:],
                                mybir.ActivationFunctionType.Copy)
                        aggTs[blk] = aggT
                    if it == 1:
                        # B-half table for THIS layer's region-1 gathers
                        nc.gpsimd.collective_compute(
                            "AllGather", AOT.bypass, replica_groups=rg,
                            ins=[h_loc[l][HA:NL, :].opt()],
                            outs=[table[l][rowsA:NP, :].opt()])
                    if it >= K:
                        # region-B gathers + finalize block it-K
                        blk = it - K
                        subs = {}
                        for (st, ln) in plan12.calls[blk][1]:
                            m = mp.tile([128, GMAX, 2 * fi], F16, tag="msg")
                            nc.gpsimd.dma_gather(
                                m[:, 0:ln, :], table[l][rowsA:, :],
                                idx12_sb[:, st * 8:(st + ln) * 8],
                                ln * 128, ln * 128, 2 * fi, elem_step=2 * fi,
                                queue_num=gq % NQ)
                            gq += 1
                            for t in range(ln):
                                subs[st + t] = (m, t)
                        us, mid, ue = plan12.ublk[blk]
                        aggT = aggTs.pop(blk)
                        if ue > mid:
                            S_sb = spB.tile([128, (ue - mid) * 128], F16,
                                            tag="SB")
                            build_S(S_sb, oo12_sb, mid, ue - mid)
                            for c4 in range(4):
                                c = blk * 4 + c4
                                ulist = plan12.units_cr.get((c, 1), [])
                                if not ulist:
                                    continue
                                pa = pa_pool.tile([fi, 128], F32, tag="pa")
                                # seed the accumulator with the region-A
                                # partial via an identity matmul, then add
                                # the region-B units; one Copy writes back.
                                nc.tensor.matmul(
                                    pa[:], ident_sb[0:fi, 0:fi],
                                    aggT[:, c4 * 128:(c4 + 1) * 128],
                                    start=True, stop=False)
                                for i, (gt, p, u) in enumerate(ulist):
                                    m, t = subs[gt]
                                    nc.tensor.matmul(
                                        pa[:], m[:, t, p * fi:(p + 1) * fi],
                                        S_sb[:, (u - mid) * 128:
                                             (u - mid + 1) * 128],
                                        start=False,
                                        stop=(i == len(ulist) - 1))
                                nc.scalar.activation(
                                    aggT[:, c4 * 128:(c4 + 1) * 128], pa[:],
                                    mybir.ActivationFunctionType.Copy)
                        finish_block(l, blk, aggT, prevT, nextT)
                        if l < 2 and blk == HA_BLOCKS + 1:
                            nc.gpsimd.collective_compute(
                                "AllGather", AOT.bypass, replica_groups=rg,
                                ins=[h_loc[l + 1][0:HA, :].opt()],
                                outs=[table[l + 1][0:rowsA, :].opt()])
            # ---- global min (via negate+max) + action mask ----
            mar = cp.tile([128, 1], F32, tag="mar")
            nc.gpsimd.partition_all_reduce(mar[:], rmax[:], 128,
                                           bass_isa.ReduceOp.max)
            nc.sync.dma_start(out=ccmin_in[:], in_=mar[0:1, :])
            nc.gpsimd.collective_compute(
                "AllGather", AOT.bypass, replica_groups=rg,
                ins=[ccmin_in[:].opt()], outs=[ccmin_out[:].opt()])
            gmaxs = cp.tile([1, cfg.n_cores], F32, tag="gmaxs")
            nc.sync.dma_start(out=gmaxs[:], in_=ccmin_out[:, 0][None, :])
            gmax = cp.tile([1, 1], F32, tag="gmax")
            nc.vector.tensor_reduce(gmax[:], gmaxs[:], mybir.AxisListType.X, AOT.max)
            gm1 = cp.tile([1, 1], F32, tag="gm1")
            nc.vector.tensor_scalar(gm1[:], gmax[:], -1.0, -1.0, AOT.mult, AOT.add)
            gm1b = cp.tile([128, 1], F32, tag="gm1b")
            nc.gpsimd.partition_broadcast(gm1b[:], gm1[:])
            repl = cp.tile([128, nch * 2], F32, tag="repl")
            nc.vector.tensor_scalar(repl[:], h3_sb[:], 0.0, gm1b[:],
                                    AOT.mult, AOT.add)
            nc.vector.copy_predicated(h3_sb[:], maskC_sb[:], repl[:])
            nc.sync.dma_start(out=out_ext[:], in_=h3_sb[:])

    nc.compile()
    return nc


def run(cfg, inputs, trace=False):
    x = np.asarray(inputs["x"], dtype=np.float32)
    src = np.asarray(inputs["src"]).astype(np.int64)
    dst = np.asarray(inputs["dst"]).astype(np.int64)
    et = np.asarray(inputs["etypes"]).astype(np.int64)
    cs = np.asarray(inputs["cell_size"]).astype(np.int64)
    ms = np.asarray(inputs["max_size"]).astype(np.int64)

    (plan0, plan12, idx0_arrs, oo0_arrs, idx12_arrs, oo12_arrs,
     xT, maskC, minmask) = preprocess(cfg, x, src, dst, et, cs, ms)
    nc = build_program(cfg, plan0, plan12)

    iota_c = np.broadcast_to(np.arange(128, dtype=np.float16), (128, 128)).copy()
    ident_c = np.eye(128, dtype=np.float16)
    common = dict(
        xdir=x.astype(np.float16), ident_c=ident_c, iota_c=iota_c,
        W1=np.asarray(inputs["W1"], np.float16),
        loop1=np.asarray(inputs["loop1"], np.float16),
        b1=np.asarray(inputs["b1"], np.float32),
        W2=np.asarray(inputs["W2"], np.float16),
        loop2=np.asarray(inputs["loop2"], np.float16),
        b2=np.asarray(inputs["b2"], np.float32),
        W3=np.asarray(inputs["W3"], np.float16),
        loop3=np.asarray(inputs["loop3"], np.float16),
        b3=np.asarray(inputs["b3"], np.float32),
    )
    in_maps = []
    for c in range(cfg.n_cores):
        m = dict(common)
        m["xT"] = xT[c]
        m["idx0"] = idx0_arrs[c]
        m["oo0"] = oo0_arrs[c]
        m["idx12"] = idx12_arrs[c]
        m["oo12"] = oo12_arrs[c]
        m["maskC"] = maskC[c]
        m["minmask"] = minmask[c]
        in_maps.append(m)

    import os as _os
    tmpdir = _os.environ.get("GNN_TRACE_DIR") or None
    nch = cfg.NLP // 128
    for attempt in range(3):
        res = run_bass_kernel_spmd(nc, in_maps, list(range(cfg.n_cores)),
                                   trace=trace, tmpdir=tmpdir)
        out = np.empty((cfg.N, 2), dtype=np.float32)
        for c in range(cfg.n_cores):
            o = res.results[c]["out"]
            o = o.reshape(128, nch, 2).transpose(1, 0, 2).reshape(cfg.NLP, 2)
            out[c * cfg.NL:(c + 1) * cfg.NL] = o[:cfg.NL]
        if np.isfinite(out).all():
            break
    return out, res


def kernel(**inputs):
    cfg = Cfg(N=50000, E=800000, feats=[128, 64, 64, 2], n_cores=8)
    out, _ = run(cfg, inputs)
    return out
